# revision 39
# baseline (speedup 1.0000x reference)
"""Trainium2 Bass kernel for Luong-style attention.

Reference computation (per full problem):
    h = decoder_hidden @ W.T + b          # [B, De]
    enc = encoder_output.transpose(1,0,2) # [B, S, De]
    a = softmax(einsum('bsd,bd->bs', enc, h), axis=1)
    context = einsum('bs,bsd->bd', a, enc)  # [B, De]

Shapes: B=64, S=4096, Dd=1024, De=512 (f32).

Strategy: data-parallel over B across 8 NeuronCores (B_local=8 each).
encoder_output is the huge tensor (512 MB); each core streams its
64 MB f32 shard from HBM exactly once over a gapless HWDGE pipeline
(~5.9us per 2MB s-tile).  Engine split, all under the DMA roofline:
  - DVE: scores via scalar_tensor_tensor (f32 product + row-sum fused,
    ~0.6us per [128,512]) against a partition-broadcast copy of h;
    per-chunk row-max.  One of the 8 score columns per tile is
    offloaded: DVE computes just the fp16 product (2x mode, ~327ns)
    and ACT does the row-sum via an activation-Copy accum_out --
    this balances DVE (~158us) vs ACT (~155us) and measured ~25us
    faster under HBM congestion than the all-DVE variant.
  - ACT: f32->fp16 cast of each tile for the context path, chunk
    softmax exp (fused row-sum accumulator), the offloaded row-sum,
    small PSUM->SBUF copies.
  - PE: score transposes, prob transposes, and the context matmuls
    (lhsT = fp16 enc d-slice as weights -> fast-weight-load ~27ns,
    rhs = fp16 prob column, N=1, LDW+MM pair ~53ns warm).
  - ALL chunk context partials accumulate into a SINGLE PSUM bank
    ([P, nchunk, M, B] = 1.25KB/partition) in one kernel-long
    accumulation group (one bank-wide zero matmul opens it, since
    start=True clears has_written bank-wide; only the final matmul
    stops it).  Nothing downstream of the PE enters the DVE/ACT
    queues during the stream, which is what keeps the DMA gapless.
  - Per-chunk max/sum are combined once at the end (softmax weights
    w_c = exp(m_c - M)/L broadcast via f16 selector matmuls).
  - W and decoder_hidden are passed in host-side pre-transposed and
    f16 (small layout/precision prep; h is computed on-device), and
    those setup loads win HBM ahead of the enc stream via an
    explicit dep -- h lands ~30us late otherwise.
  - The last two tiles' loads are split by b-pairs so their score
    ops pipeline against the straggling SDMA engine (~10% slower),
    and the chunk schedule ends with short chunks to shrink the
    drain tail.
No collectives needed.  ~215-225us on silicon vs ~187us single-pass
HBM roofline (64 MB/core at ~358 GB/s); rel err ~1e-3.
"""

import numpy as np

import concourse.bass as bass
import concourse.bacc as bacc_mod
import concourse.tile as tile
import concourse.mybir as mybir
from concourse import masks
from concourse.bass_utils import run_bass_kernel_spmd

F32 = mybir.dt.float32
F16 = mybir.dt.float16
ALU = mybir.AluOpType
ACTF = mybir.ActivationFunctionType
AX = mybir.AxisListType

NCORES = 8
B = 8          # per-core batch
S = 4096
DD = 1024
DE = 512
P = 128        # s-values per tile
M = DE // P              # 4 d-chunks
NTILES = S // P          # 32
CHUNKS = [4, 4, 4, 4, 4, 4, 4, 2, 1, 1]   # tiles per softmax chunk
assert sum(CHUNKS) == NTILES
CMAX = max(CHUNKS)

# "swdge_cast": stream enc with inline f32->f16 cast on the gpsimd DMA path.
#   (measured: the single SWDGE context serializes ~2us of completion latency
#   per DMA -> stream runs at ~280 GB/s. kept only for reference)
# "hwdge": plain f32 HWDGE stream (sustains ~360 GB/s); scores read the f32
#   tile directly on DVE, ACT casts to fp16 for the context-matmul path.
DMA_MODE = "hwdge"


def build_nc():
    nchunk = len(CHUNKS)
    nc = bacc_mod.Bacc("TRN2", target_bir_lowering=False, debug=False)
    decT_d = nc.dram_tensor("decT", [DD, B], F16, kind="ExternalInput")
    enc_d = nc.dram_tensor("encoder_output", [S, B, DE], F32, kind="ExternalInput")
    wt_d = nc.dram_tensor("WT", [DD, DE], F16, kind="ExternalInput")
    b_d = nc.dram_tensor("b", [DE], F32, kind="ExternalInput")
    out_d = nc.dram_tensor("out", [B, DE], F32, kind="ExternalOutput")

    with tile.TileContext(nc) as tc:
        with (
            tc.tile_pool(name="const", bufs=1) as const_pool,
            tc.tile_pool(name="persist", bufs=1) as persist_pool,
            tc.tile_pool(name="enc16", bufs=8) as enc16_pool,
            tc.tile_pool(name="enc32", bufs=5) as enc32_pool,
            tc.tile_pool(name="scratch", bufs=3) as scratch_pool,
            tc.tile_pool(name="sct", bufs=12) as sct_pool,
            tc.tile_pool(name="probs", bufs=3) as p_pool,
            tc.tile_pool(name="pt", bufs=6) as pt_pool,
            tc.tile_pool(name="stat", bufs=6) as stat_pool,
        ):
            # ---------------- constants ----------------
            ident = const_pool.tile([P, P], F32)
            masks.make_identity(nc, ident[:])
            ones_f32 = const_pool.tile([1, P], F32)
            nc.vector.memset(ones_f32[:], 1.0)
            ones16 = const_pool.tile([1, P], F16)
            nc.vector.memset(ones16[:], 1.0)
            # row-broadcast selectors: sel[:, bb, :] is [8, 128] with row bb
            # all-ones; matmul(sel_bb, x) broadcasts x's row bb to all
            # 128 partitions without any cross-partition DMA.
            sel = const_pool.tile([B, B, P], F32)
            nc.gpsimd.memset(sel[:], 0.0)
            nc.gpsimd.affine_select(
                out=sel[:], in_=sel[:],
                compare_op=ALU.not_equal, fill=1.0, base=0,
                pattern=[[-1, B], [0, P]], channel_multiplier=1)
            # f16 copy of the selectors: the end-combine broadcast matmuls
            # load them as weights, and f16 weights get fast-weight-load
            sel16 = const_pool.tile([B, B, P], F16)
            nc.vector.tensor_copy(sel16[:], sel[:])

            # ---------------- persistent state ----------------
            # hb matches the dtype the score STT reads (f32 tiles in hwdge
            # mode, f16 tiles in swdge_cast mode)
            hb = persist_pool.tile([P, B, DE],
                                   F16 if DMA_MODE == "swdge_cast" else F32)
            nchunk = len(CHUNKS)
            m_all = persist_pool.tile([B, nchunk], F32)
            negm_all = persist_pool.tile([B, nchunk], F32)
            l_all = persist_pool.tile([B, nchunk], F32)

            # ---------------- setup: h = dec @ W.T + b ----------------
            setup_cm = tc.tile_pool(name="setup_sb", bufs=1)
            setup_sb = setup_cm.__enter__()
            psum_setup_cm = tc.tile_pool(name="psum_setup", bufs=2, space="PSUM")
            psum_setup = psum_setup_cm.__enter__()

            # W / dec arrive as f16 (host-cast): half the setup bytes that
            # gate the stream start, and f16 weights matmul with FWL.
            decT_sb = setup_sb.tile([P, DD // P, B], F16)
            wt_sb = setup_sb.tile([P, DD // P, DE], F16)
            bias_sb = setup_sb.tile([1, DE], F32)
            h_sb = setup_sb.tile([B, DE], F32)
            # issue setup loads on the Activation HWDGE queue as 3 batched
            # DMAs (HWDGE dispatch is ~700ns per instruction, so many small
            # DMAs serialize); the first enc DMA is made to depend on them
            # (below) so the small W/dec loads win HBM before the 2MB enc
            # tiles flood the SDMA engines -- otherwise h/hb lands ~30us
            # late and the whole pipeline starts (and stays) that far behind.
            setup_dmas = [
                nc.scalar.dma_start(
                    decT_sb[:], decT_d.rearrange("(k p) b -> p k b", p=P)),
                nc.scalar.dma_start(bias_sb[:], b_d[None, :]),
            ]
            for kc in range(DD // P):
                setup_dmas.append(nc.scalar.dma_start(
                    wt_sb[:, kc, :], wt_d[kc * P:(kc + 1) * P, :]))

            h_ps = psum_setup.tile([B, DE], F32, tag="su")
            for kc in range(DD // P):
                nc.tensor.matmul(h_ps[:], decT_sb[:, kc, :], wt_sb[:, kc, :],
                                 start=(kc == 0), stop=False)
            nc.tensor.matmul(h_ps[:], ones_f32[0:1, 0:B], bias_sb[:],
                             start=False, stop=True)
            nc.vector.tensor_copy(h_sb[:], h_ps[:])

            # broadcast h along partitions into hb via selector matmuls;
            # the last b also gets an fp16 copy for the offloaded score path
            hb16 = persist_pool.tile([P, DE], F16)
            for bb in range(B):
                hp = psum_setup.tile([P, DE], F32, tag="su")
                nc.tensor.matmul(hp[:], sel[:, bb, :], h_sb[:],
                                 start=True, stop=True)
                nc.vector.tensor_copy(hb[:, bb, :], hp[:])
                if bb == B - 1:
                    nc.scalar.copy(hb16[:], hp[:])

            psum_setup_cm.__exit__(None, None, None)
            setup_cm.__exit__(None, None, None)

            # ---------------- main loop PSUM pools ----------------
            sc_cm = tc.tile_pool(name="psum_sc", bufs=3, space="PSUM")
            psum_sc = sc_cm.__enter__()
            tr_cm = tc.tile_pool(name="psum_tr", bufs=3, space="PSUM")
            psum_tr = tr_cm.__enter__()
            ctx_cm = tc.tile_pool(name="psum_ctx", bufs=1, space="PSUM")
            psum_ctx = ctx_cm.__enter__()

            # ALL chunk context partials live in one PSUM bank for the whole
            # stream (each is only [P, M, B] = 128B/partition).  One zero
            # matmul opens the bank-wide accumulation group; every context
            # matmul accumulates with start=False; only the very last one
            # stops the group.  Nothing downstream of the PE ever enters the
            # DVE/ACT queues during the stream.
            ctx_ps = psum_ctx.tile([P, nchunk, M, B], F32)
            zrow16 = const_pool.tile([1, nchunk * M * B], F16)
            nc.vector.memset(zrow16[:], 0.0)
            nc.tensor.matmul(ctx_ps[:], ones16[:], zrow16[:],
                             start=True, stop=False)

            jglobal = 0
            for ci, ct in enumerate(CHUNKS):
                scT = psum_sc.tile([B, CMAX * P], F32)
                ets = []
                for t in range(ct):
                    j = jglobal + t
                    et = enc16_pool.tile([P, B, DE], F16)
                    if DMA_MODE == "swdge_cast":
                        nc.gpsimd.dma_start(et[:], enc_d[j * P:(j + 1) * P, :, :])
                        score_src = et
                    else:
                        et32 = enc32_pool.tile([P, B, DE], F32)
                        if j >= NTILES - 4:
                            # last tiles: split the load by b-pairs so the
                            # score STTs pipeline against the tile's own
                            # arrival instead of waiting for the full 2MB
                            # (the slowest SDMA engine runs ~10% behind).
                            for k in range(4):
                                nc.sync.dma_start(
                                    et32[:, 2 * k:2 * k + 2, :],
                                    enc_d[j * P:(j + 1) * P, 2 * k:2 * k + 2, :])
                            # split the fp16 cast the same way so it also
                            # pipelines against the sub-loads -- a monolithic
                            # 3.7us cast sits on the critical drain tail
                            for k in range(4):
                                nc.scalar.copy(et[:, 2 * k:2 * k + 2, :],
                                               et32[:, 2 * k:2 * k + 2, :])
                        else:
                            enc_dma = nc.sync.dma_start(
                                et32[:], enc_d[j * P:(j + 1) * P, :, :])
                            if j == 0:
                                for sd in setup_dmas:
                                    tile.add_dep_helper(
                                        enc_dma.ins, sd.ins,
                                        reason="let setup W loads win HBM first")
                            nc.scalar.copy(et[:], et32[:])
                        score_src = et32
                    ets.append(et)
                    sct = sct_pool.tile([P, B], F32, tag="sct")
                    for bb in range(B - 1):
                        junk = scratch_pool.tile([P, DE], F16, tag="junk")
                        nc.vector.scalar_tensor_tensor(
                            out=junk[:],
                            in0=score_src[:, bb, :],
                            scalar=1.0,
                            in1=hb[:, bb, :],
                            op0=ALU.mult,
                            op1=ALU.mult,
                            accum_out=sct[:, bb:bb + 1],
                        )
                    prod16 = scratch_pool.tile([P, DE], F16, tag="prod")
                    nc.vector.tensor_tensor(
                        out=prod16[:], in0=et[:, B - 1, :], in1=hb16[:],
                        op=ALU.mult)
                    junka = scratch_pool.tile([P, DE], F16, tag="junka")
                    nc.scalar.activation(junka[:], prod16[:], ACTF.Copy,
                                         accum_out=sct[:, B - 1:B])
                    nc.tensor.transpose(scT[:, t * P:(t + 1) * P], sct[:], ident[:])
                jglobal += ct

                # ---- chunk softmax stats ----
                nc.vector.reduce_max(m_all[:, ci:ci + 1], scT[:, :ct * P],
                                     axis=AX.X)
                nc.vector.tensor_scalar_mul(negm_all[:, ci:ci + 1],
                                            m_all[:, ci:ci + 1], -1.0)
                # p = exp(scores - m_c), l_c = row-sum
                p_sb = p_pool.tile([B, CMAX * P], F32)
                nc.scalar.activation(p_sb[:, :ct * P], scT[:, :ct * P], ACTF.Exp,
                                     bias=negm_all[:, ci:ci + 1], scale=1.0,
                                     accum_out=l_all[:, ci:ci + 1])

                # ---- context partial for this chunk into its PSUM slot ----
                for t in range(ct):
                    ptp = psum_tr.tile([P, B], F32, tag="tr")
                    nc.tensor.transpose(ptp[:], p_sb[:, t * P:(t + 1) * P],
                                        ident[0:B, 0:B])
                    pts = pt_pool.tile([P, B], F16)
                    nc.scalar.copy(pts[:], ptp[:])
                    for bb in range(B):
                        for mm in range(M):
                            last = (ci == nchunk - 1 and t == ct - 1
                                    and bb == B - 1 and mm == M - 1)
                            nc.tensor.matmul(
                                ctx_ps[:, ci, mm, bb:bb + 1],
                                ets[t][:, bb, mm * P:(mm + 1) * P],
                                pts[:, bb:bb + 1],
                                start=False, stop=last)

            # ---------------- end combine over chunks ----------------
            g_max = stat_pool.tile([B, 1], F32, tag="gmax")
            g_negmax = stat_pool.tile([B, 1], F32, tag="gneg")
            g_l = stat_pool.tile([B, 1], F32, tag="gl")
            g_rl = stat_pool.tile([B, 1], F32, tag="grl")
            w_all = stat_pool.tile([B, nchunk], F32, tag="wall")
            nc.vector.reduce_max(g_max[:], m_all[:], axis=AX.X)
            nc.vector.tensor_scalar_mul(g_negmax[:], g_max[:], -1.0)
            nc.scalar.activation(w_all[:], m_all[:], ACTF.Exp,
                                 bias=g_negmax[:], scale=1.0)
            junk2 = stat_pool.tile([B, nchunk], F32, tag="junk2")
            nc.vector.scalar_tensor_tensor(
                out=junk2[:], in0=l_all[:], scalar=1.0, in1=w_all[:],
                op0=ALU.mult, op1=ALU.mult, accum_out=g_l[:])
            nc.vector.reciprocal(g_rl[:], g_l[:])
            # normalized chunk weights wn[b, c] = w[b, c] / l_total[b]
            w_norm = stat_pool.tile([B, nchunk], F16, tag="wnorm")
            nc.vector.tensor_scalar(out=w_norm[:], in0=w_all[:],
                                    scalar1=g_rl[:, 0:1], scalar2=None,
                                    op0=ALU.mult)
            # broadcast wn along partitions: [P, nchunk, b] via selector matmuls
            wb = persist_pool.tile([P, nchunk, B], F32)
            for bb in range(B):
                wbp = psum_tr.tile([P, nchunk], F32, tag="tr")
                nc.tensor.matmul(wbp[:], sel16[:, bb, :], w_norm[:],
                                 start=True, stop=True)
                nc.scalar.copy(wb[:, :, bb], wbp[:])
            # weighted sum over chunks (still transposed): [P, m, b]
            ctxf = persist_pool.tile([P, M, B], F32)
            tmpw = persist_pool.tile([P, nchunk, B], F32)
            out_sb = persist_pool.tile([B, DE], F32)
            for mm in range(M):
                nc.vector.tensor_tensor(out=tmpw[:], in0=ctx_ps[:, :, mm, :],
                                        in1=wb[:], op=ALU.mult)
                nc.vector.reduce_sum(
                    ctxf[:, mm, :],
                    tmpw[:].rearrange("p c b -> p b c"),
                    axis=AX.X)
                op_ps = psum_tr.tile([B, P], F32, tag="tr")
                nc.tensor.transpose(op_ps[:], ctxf[:, mm, :], ident[:])
                nc.vector.tensor_copy(out_sb[:, mm * P:(mm + 1) * P], op_ps[:])
            nc.sync.dma_start(out_d[:], out_sb[:])

            ctx_cm.__exit__(None, None, None)
            tr_cm.__exit__(None, None, None)
            sc_cm.__exit__(None, None, None)

    nc.compile()
    if not nc.is_finalized():
        nc.finalize()
    return nc


_NC = None


def make_in_maps(decoder_hidden, encoder_output, W, b):
    decoder_hidden = np.ascontiguousarray(decoder_hidden, dtype=np.float32)
    encoder_output = np.ascontiguousarray(encoder_output, dtype=np.float32)
    WT = np.ascontiguousarray(np.asarray(W, dtype=np.float32).T)
    b = np.ascontiguousarray(b, dtype=np.float32)
    WT16 = WT.astype(np.float16)
    in_maps = []
    for i in range(NCORES):
        sl = slice(i * B, (i + 1) * B)
        in_maps.append({
            "decT": np.ascontiguousarray(decoder_hidden[sl].T.astype(np.float16)),
            "encoder_output": np.ascontiguousarray(encoder_output[:, sl, :]),
            "WT": WT16,
            "b": b,
        })
    return in_maps


def kernel(decoder_hidden, encoder_output, W, b):
    global _NC
    if _NC is None:
        _NC = build_nc()
    in_maps = make_in_maps(decoder_hidden, encoder_output, W, b)
    res = run_bass_kernel_spmd(_NC, in_maps, core_ids=list(range(NCORES)))
    return np.concatenate([res.results[i]["out"] for i in range(NCORES)], axis=0)
